# revision 8
# baseline (speedup 1.0000x reference)
"""MoE top-2 routing kernel for 8 TRN2 NeuronCores (expert parallelism).

Strategy: the router (8192x1024 @ 1024x8 + softmax + top-2) is computed on
host in fp32; tokens are dispatched to the core that owns their expert
(expert e -> core e).  Each core runs the expert MLP
    y = (silu(x @ G) * (x @ U)) @ Dw   scaled per-row by the combine weight
over its padded token batch in bf16 (fp32 PSUM accumulation).  Host
scatter-adds the per-expert outputs back into the [B,S,D] output.

Layout per core (SBUF-resident unless noted):
  gw, uw [1024, 4096] bf16    lhsT for the gate/up matmuls (K=D on partitions)
  xt     [1024, C]    bf16    gathered tokens, transposed (streamed per group)
  dw     [4096, 1024] bf16    down weights (streamed per token group)
  h      [4096, gw]   bf16    silu(gate)*up, transposed (inter on partitions)
                              = exactly the lhsT layout the down matmul needs
Token groups of 512 keep every matmul at N=512 (or 256 for the tail group).
"""

import functools
import sys
import types
from contextlib import ExitStack

import numpy as np
import ml_dtypes

import concourse.bass as bass
import concourse.tile as tile
import concourse.mybir as mybir
from concourse.vector_clock import ScopedClock
from concourse.bass_utils import run_bass_kernel_spmd

# ---------------------------------------------------------------------------
# problem constants (hardcoded per contract)
# ---------------------------------------------------------------------------
NUM_EXPERTS = 8
TOP_K = 2
HIDDEN = 1024          # D
INTER = 4096           # I
B, S = 4, 2048
T = B * S              # 8192 tokens
N_CORES = 8

DK = HIDDEN // 128     # 8  k-tiles over D
IM = INTER // 128      # 32 m-tiles over I

BF16 = mybir.dt.bfloat16
F32 = mybir.dt.float32

# BassKernelResults of the most recent device run (for test harnesses)
last_results = None

# walrus in this container rejects >1 sync wait per instruction; Tile's
# kernel-tail drain aggregates one wait per active proc.  Split them across
# preceding same-engine NOPs.
_MAX_WAITS = 1


def _patched_drain_and_barrier(self, tick_clock, wait_clock):
    nc = self.nc
    probe = nc.sync.nop(nofuse=True)
    wait_clock.add_sem_waits(probe.ins, ScopedClock({None: tick_clock.global_clock}))
    si = probe.ins.sync_info
    waits = list(si.on_wait) if si is not None else []
    if si is not None:
        si.on_wait[:] = waits[:_MAX_WAITS]
    rest = waits[_MAX_WAITS:]
    while rest:
        nop = nc.sync.nop(nofuse=True)
        nop.ins.sync_info = mybir.SyncInfo(on_wait=rest[:_MAX_WAITS], on_update=[])
        rest = rest[_MAX_WAITS:]
    nc.sync.drain()
    nc.all_engine_barrier()
    assert self.sems is not None
    popped = nc._tile_sem_poison_stack.pop()
    assert popped is self._sem_poison
    nc.clear_and_free_semaphores(list(self.sems.allocated().values()))
    nc.all_engine_barrier()


tile.TileContext._drain_and_barrier = _patched_drain_and_barrier


def _split_multi_waits(nc):
    """Safety net: hoist extra sync waits from any instruction onto
    preceding same-engine NOPs (walrus here allows 1 wait per inst)."""
    n = 0
    for f in nc.m.functions:
        for bb in f.blocks:
            new = []
            for ins in bb.instructions:
                si = getattr(ins, "sync_info", None)
                if si is not None and len(si.on_wait) > _MAX_WAITS:
                    waits = list(si.on_wait)
                    rest = waits[_MAX_WAITS:]
                    si.on_wait[:] = waits[:_MAX_WAITS]
                    while rest:
                        n += 1
                        nop = mybir.InstNoOp(
                            name=f"waitsplit-{n}",
                            engine=ins.engine,
                            ins=[],
                            outs=[],
                            sync_info=mybir.SyncInfo(
                                on_wait=rest[:_MAX_WAITS], on_update=[]
                            ),
                        )
                        new.append(nop)
                        rest = rest[_MAX_WAITS:]
                new.append(ins)
            if n:
                bb.instructions[:] = new
    return n


def _token_groups(c):
    groups = []
    off = 0
    while off < c:
        w = min(512, c - off)
        groups.append((off, w))
        off += w
    return groups


def build_expert_kernel(c_tokens):
    """One expert's MLP over c_tokens padded tokens (SPMD across 8 cores)."""
    nc = bass.Bass()
    xt_d = nc.dram_tensor("xt", [HIDDEN, c_tokens], BF16, kind="ExternalInput")
    gw_d = nc.dram_tensor("gw", [HIDDEN, INTER], BF16, kind="ExternalInput")
    uw_d = nc.dram_tensor("uw", [HIDDEN, INTER], BF16, kind="ExternalInput")
    dw_d = nc.dram_tensor("dw", [INTER, HIDDEN], BF16, kind="ExternalInput")
    cw_d = nc.dram_tensor("cw", [c_tokens, 1], F32, kind="ExternalInput")
    y_d = nc.dram_tensor("y", [c_tokens, HIDDEN], F32, kind="ExternalOutput")

    groups = _token_groups(c_tokens)
    n_tok_tiles = c_tokens // 128

    with ExitStack() as ctx:
        tc = ctx.enter_context(tile.TileContext(nc))
        wpool = ctx.enter_context(tc.tile_pool(name="weights", bufs=1))
        dwpool = ctx.enter_context(tc.tile_pool(name="dw", bufs=4))
        xpool = ctx.enter_context(tc.tile_pool(name="xt", bufs=2))
        hpool = ctx.enter_context(tc.tile_pool(name="h", bufs=1))
        spool = ctx.enter_context(tc.tile_pool(name="s", bufs=4))
        opool = ctx.enter_context(tc.tile_pool(name="out", bufs=3))
        # one shared 8-slot PSUM pool: phase A cycles gate/up pairs through
        # it (2 live), phase B holds all 8 down-accumulators at once
        pspool = ctx.enter_context(tc.tile_pool(name="ps", bufs=8, space="PSUM"))

        # resident weights: gate/up as [128, DK*INTER] (k-tile k at col k*INTER)
        gw_sb = wpool.tile([128, DK * INTER], BF16, tag="gw")
        uw_sb = wpool.tile([128, DK * INTER], BF16, tag="uw")
        for k in range(DK):
            nc.sync.dma_start(
                gw_sb[:, k * INTER:(k + 1) * INTER], gw_d[k * 128:(k + 1) * 128, :]
            )
            nc.sync.dma_start(
                uw_sb[:, k * INTER:(k + 1) * INTER], uw_d[k * 128:(k + 1) * 128, :]
            )
        # combine weights: column t = token tile t, [128, 1] each
        cw_sb = wpool.tile([128, n_tok_tiles], F32, tag="cw")
        for t in range(n_tok_tiles):
            nc.sync.dma_start(cw_sb[:, t:t + 1], cw_d[t * 128:(t + 1) * 128, :])

        for goff, gwid in groups:
            # ---- load x^T for this token group: k-tile k at col k*512 ----
            xt_sb = xpool.tile([128, DK * 512], BF16, tag="xt")
            for k in range(DK):
                nc.sync.dma_start(
                    xt_sb[:, k * 512:k * 512 + gwid],
                    xt_d[k * 128:(k + 1) * 128, goff:goff + gwid],
                )

            # ---- phase A: h^T[m] = silu(G^T x) * (U^T x), inter on partitions
            h_sb = hpool.tile([128, IM * 512], BF16, tag="h")
            for m in range(IM):
                pg = pspool.tile([128, gwid], F32, tag="ps")
                pu = pspool.tile([128, gwid], F32, tag="ps")
                for k in range(DK):
                    nc.tensor.matmul(
                        pg[:],
                        gw_sb[:, k * INTER + m * 128:k * INTER + (m + 1) * 128],
                        xt_sb[:, k * 512:k * 512 + gwid],
                        start=(k == 0),
                        stop=(k == DK - 1),
                    )
                for k in range(DK):
                    nc.tensor.matmul(
                        pu[:],
                        uw_sb[:, k * INTER + m * 128:k * INTER + (m + 1) * 128],
                        xt_sb[:, k * 512:k * 512 + gwid],
                        start=(k == 0),
                        stop=(k == DK - 1),
                    )
                s_sb = spool.tile([128, gwid], F32, tag="s")
                nc.scalar.activation(
                    s_sb[:], pg[:], mybir.ActivationFunctionType.Silu
                )
                nc.vector.tensor_mul(
                    h_sb[:, m * 512:m * 512 + gwid], s_sb[:], pu[:]
                )

            # ---- phase B: y[tok] = h^T.T @ Dw, scaled by combine weight ----
            # m outermost so each streamed dw tile is consumed and released;
            # all (half, t) accumulators live across the m loop.
            n_t = gwid // 128
            py_tiles = [
                [
                    pspool.tile([128, 512], F32, tag="ps", name=f"py_{half}_{t}")
                    for t in range(n_t)
                ]
                for half in range(2)
            ]
            for m in range(IM):
                dwt = dwpool.tile([128, HIDDEN], BF16, tag="dw")
                nc.sync.dma_start(dwt[:], dw_d[m * 128:(m + 1) * 128, :])
                for half in range(2):
                    for t in range(n_t):
                        nc.tensor.matmul(
                            py_tiles[half][t][:],
                            h_sb[:, m * 512 + t * 128:m * 512 + (t + 1) * 128],
                            dwt[:, half * 512:(half + 1) * 512],
                            start=(m == 0),
                            stop=(m == IM - 1),
                        )
            for t in range(n_t):
                out_sb = opool.tile([128, HIDDEN], F32, tag="out")
                tile_idx = goff // 128 + t
                for half in range(2):
                    nc.vector.tensor_scalar_mul(
                        out_sb[:, half * 512:(half + 1) * 512],
                        py_tiles[half][t][:],
                        cw_sb[:, tile_idx:tile_idx + 1],
                    )
                nc.sync.dma_start(
                    y_d[tile_idx * 128:(tile_idx + 1) * 128, :], out_sb[:]
                )

    _split_multi_waits(nc)
    return nc


@functools.lru_cache(maxsize=2)
def _compiled_ncs(c_tokens):
    return build_expert_kernel(c_tokens)


def _route_host(x, router_w):
    """fp32 host routing identical to the jax reference."""
    logits = x @ router_w                                     # [T, E]
    m = logits.max(axis=-1, keepdims=True)
    p = np.exp(logits - m)
    p /= p.sum(axis=-1, keepdims=True)
    idx = np.argsort(-p, axis=-1, kind="stable")[:, :TOP_K]   # [T, K]
    w = np.take_along_axis(p, idx, axis=-1)
    w = w / w.sum(axis=-1, keepdims=True)
    return p, idx, w


def kernel(hidden_states, router_w, gate_w, up_w, down_w):
    hidden_states = np.asarray(hidden_states, dtype=np.float32)
    router_w = np.asarray(router_w, dtype=np.float32)
    x = hidden_states.reshape(T, HIDDEN)

    probs, idx, w = _route_host(x, router_w)

    # load-balance loss (host, fp32, matches reference formula)
    counts = np.bincount(idx.ravel(), minlength=NUM_EXPERTS)
    expert_usage = counts.astype(np.float32) / np.float32(T)
    mean_probs = probs.mean(axis=0, dtype=np.float32)
    loss = np.float32((expert_usage * mean_probs).sum() * NUM_EXPERTS)

    # capacity: padded max expert load (deterministic inputs -> 2304)
    c_tokens = max(2304, int(-(-counts.max() // 128) * 128))

    xb = x.astype(ml_dtypes.bfloat16)
    gwb = np.asarray(gate_w).astype(ml_dtypes.bfloat16)
    uwb = np.asarray(up_w).astype(ml_dtypes.bfloat16)
    dwb = np.asarray(down_w).astype(ml_dtypes.bfloat16)

    in_maps = []
    tok_lists = []
    for e in range(NUM_EXPERTS):
        tok_e, k_e = np.nonzero(idx == e)
        n_e = len(tok_e)
        tok_lists.append((tok_e, n_e))
        xt = np.zeros((HIDDEN, c_tokens), dtype=ml_dtypes.bfloat16)
        xt[:, :n_e] = xb[tok_e].T
        cw = np.zeros((c_tokens, 1), dtype=np.float32)
        cw[:n_e, 0] = w[tok_e, k_e]
        in_maps.append(
            {
                "xt": xt,
                "gw": np.ascontiguousarray(gwb[e]),
                "uw": np.ascontiguousarray(uwb[e]),
                "dw": np.ascontiguousarray(dwb[e]),
                "cw": cw,
            }
        )

    nc = _compiled_ncs(c_tokens)
    res = run_bass_kernel_spmd(nc, in_maps, core_ids=list(range(N_CORES)))
    global last_results
    last_results = res

    out = np.zeros((T, HIDDEN), dtype=np.float32)
    for e in range(NUM_EXPERTS):
        tok_e, n_e = tok_lists[e]
        out[tok_e] += res.results[e]["y"][:n_e]

    return out.reshape(B, S, HIDDEN), loss


# revision 17
# speedup vs baseline: 1.0747x; 1.0747x over previous
"""MoE top-2 routing kernel for 8 TRN2 NeuronCores (expert parallelism).

Strategy: the router (8192x1024 @ 1024x8 + softmax + top-2) is computed on
host in fp32; tokens are dispatched to the core that owns their expert
(expert e -> core e).  Each core runs the expert MLP
    y = (silu(x @ G) * (x @ U)) @ Dw   scaled per-row by the combine weight
over its padded token batch in bf16 (fp32 PSUM accumulation).  Host
scatter-adds the per-expert outputs back into the [B,S,D] output.

Layout per core (SBUF-resident unless noted):
  gw, uw [1024, 4096] bf16    lhsT for the gate/up matmuls (K=D on partitions)
  xt     [1024, C]    bf16    gathered tokens, transposed (streamed per group)
  dw     [4096, 1024] bf16    down weights (streamed per token group)
  h      [4096, gw]   bf16    silu(gate)*up, transposed (inter on partitions)
                              = exactly the lhsT layout the down matmul needs
Token groups of 512 keep every matmul at N=512 (or 256 for the tail group).
"""

import functools
import sys
import types
from contextlib import ExitStack

import numpy as np
import ml_dtypes

import concourse.bass as bass
import concourse.tile as tile
import concourse.mybir as mybir
from concourse.vector_clock import ScopedClock
from concourse.bass_utils import run_bass_kernel_spmd

# ---------------------------------------------------------------------------
# problem constants (hardcoded per contract)
# ---------------------------------------------------------------------------
NUM_EXPERTS = 8
TOP_K = 2
HIDDEN = 1024          # D
INTER = 4096           # I
B, S = 4, 2048
T = B * S              # 8192 tokens
N_CORES = 8

DK = HIDDEN // 128     # 8  k-tiles over D
IM = INTER // 128      # 32 m-tiles over I

BF16 = mybir.dt.bfloat16
F32 = mybir.dt.float32

# BassKernelResults of the most recent device run (for test harnesses)
last_results = None

# walrus in this container rejects >1 sync wait per instruction; Tile's
# kernel-tail drain aggregates one wait per active proc.  Split them across
# preceding same-engine NOPs.
_MAX_WAITS = 1


def _patched_drain_and_barrier(self, tick_clock, wait_clock):
    nc = self.nc
    probe = nc.sync.nop(nofuse=True)
    wait_clock.add_sem_waits(probe.ins, ScopedClock({None: tick_clock.global_clock}))
    si = probe.ins.sync_info
    waits = list(si.on_wait) if si is not None else []
    if si is not None:
        si.on_wait[:] = waits[:_MAX_WAITS]
    rest = waits[_MAX_WAITS:]
    while rest:
        nop = nc.sync.nop(nofuse=True)
        nop.ins.sync_info = mybir.SyncInfo(on_wait=rest[:_MAX_WAITS], on_update=[])
        rest = rest[_MAX_WAITS:]
    nc.sync.drain()
    nc.all_engine_barrier()
    assert self.sems is not None
    popped = nc._tile_sem_poison_stack.pop()
    assert popped is self._sem_poison
    nc.clear_and_free_semaphores(list(self.sems.allocated().values()))
    nc.all_engine_barrier()


tile.TileContext._drain_and_barrier = _patched_drain_and_barrier


def _split_multi_waits(nc):
    """Safety net: hoist extra sync waits from any instruction onto
    preceding same-engine NOPs (walrus here allows 1 wait per inst)."""
    n = 0
    for f in nc.m.functions:
        for bb in f.blocks:
            new = []
            for ins in bb.instructions:
                si = getattr(ins, "sync_info", None)
                if si is not None and len(si.on_wait) > _MAX_WAITS:
                    waits = list(si.on_wait)
                    rest = waits[_MAX_WAITS:]
                    si.on_wait[:] = waits[:_MAX_WAITS]
                    while rest:
                        n += 1
                        nop = mybir.InstNoOp(
                            name=f"waitsplit-{n}",
                            engine=ins.engine,
                            ins=[],
                            outs=[],
                            sync_info=mybir.SyncInfo(
                                on_wait=rest[:_MAX_WAITS], on_update=[]
                            ),
                        )
                        new.append(nop)
                        rest = rest[_MAX_WAITS:]
                new.append(ins)
            if n:
                bb.instructions[:] = new
    return n


def _token_groups(c):
    groups = []
    off = 0
    while off < c:
        w = min(512, c - off)
        groups.append((off, w))
        off += w
    return groups


def build_expert_kernel(c_tokens):
    """One expert's MLP over c_tokens padded tokens (SPMD across 8 cores)."""
    nc = bass.Bass()
    xt_d = nc.dram_tensor("xt", [HIDDEN, c_tokens], BF16, kind="ExternalInput")
    # gate/up weights pre-blocked host-side into [INTER//512, HIDDEN, 512]
    # so each (block, k) DMA reads contiguous rows
    gw_d = nc.dram_tensor("gw", [INTER // 512 * HIDDEN, 512], BF16, kind="ExternalInput")
    uw_d = nc.dram_tensor("uw", [INTER // 512 * HIDDEN, 512], BF16, kind="ExternalInput")
    dw_d = nc.dram_tensor("dw", [INTER, HIDDEN], BF16, kind="ExternalInput")
    cw_d = nc.dram_tensor("cw", [c_tokens, 1], F32, kind="ExternalInput")
    y_d = nc.dram_tensor("y", [c_tokens, HIDDEN], F32, kind="ExternalOutput")

    groups = _token_groups(c_tokens)
    n_tok_tiles = c_tokens // 128

    with ExitStack() as ctx:
        tc = ctx.enter_context(tile.TileContext(nc))
        wpool = ctx.enter_context(tc.tile_pool(name="weights", bufs=1))
        dwpool = ctx.enter_context(tc.tile_pool(name="dw", bufs=8))
        xpool = ctx.enter_context(tc.tile_pool(name="xt", bufs=2))
        hpool = ctx.enter_context(tc.tile_pool(name="h", bufs=1))
        spool = ctx.enter_context(tc.tile_pool(name="s", bufs=3))
        opool = ctx.enter_context(tc.tile_pool(name="out", bufs=2))
        # one shared 8-slot PSUM pool: phase A cycles gate/up pairs through
        # it (2 live), phase B holds all 8 down-accumulators at once
        pspool = ctx.enter_context(tc.tile_pool(name="ps", bufs=8, space="PSUM"))

        # resident gate/up weights, split into 512-column blocks (separate
        # tiles) so phase A's first m-tiles only depend on block 0's DMA.
        # block b tile layout: k-tile k at col k*512, covering inter columns
        # [b*512, (b+1)*512) of the original [HIDDEN, INTER] weight.
        n_blk = INTER // 512  # 8
        gw_blks = [
            wpool.tile([128, DK * 512], BF16, tag=f"gw{b}", name=f"gwb{b}")
            for b in range(n_blk)
        ]
        uw_blks = [
            wpool.tile([128, DK * 512], BF16, tag=f"uw{b}", name=f"uwb{b}")
            for b in range(n_blk)
        ]

        def load_wblock(b):
            for k in range(DK):
                r0 = b * HIDDEN + k * 128
                nc.sync.dma_start(
                    gw_blks[b][:, k * 512:(k + 1) * 512], gw_d[r0:r0 + 128, :]
                )
                nc.sync.dma_start(
                    uw_blks[b][:, k * 512:(k + 1) * 512], uw_d[r0:r0 + 128, :]
                )

        # group 0's xt interleaved with gate block 0 so the first
        # accumulation group can start after a couple of DMAs
        xt_first = xpool.tile([128, DK * 512], BF16, tag="xt", name="xt_first")
        g0off, g0wid = groups[0]
        for k in range(DK):
            nc.sync.dma_start(
                xt_first[:, k * 512:k * 512 + g0wid],
                xt_d[k * 128:(k + 1) * 128, g0off:g0off + g0wid],
            )
            r0 = k * 128
            nc.sync.dma_start(
                gw_blks[0][:, k * 512:(k + 1) * 512], gw_d[r0:r0 + 128, :]
            )
            nc.sync.dma_start(
                uw_blks[0][:, k * 512:(k + 1) * 512], uw_d[r0:r0 + 128, :]
            )
        for b in range(1, n_blk):
            load_wblock(b)
        # combine weights: column t = token tile t, [128, 1] each (tiny)
        cw_sb = wpool.tile([128, n_tok_tiles], F32, tag="cw")
        for t in range(n_tok_tiles):
            nc.sync.dma_start(cw_sb[:, t:t + 1], cw_d[t * 128:(t + 1) * 128, :])

        for gi, (goff, gwid) in enumerate(groups):
            # ---- load x^T for this token group: k-tile k at col k*512 ----
            if gi == 0:
                xt_sb = xt_first
            else:
                xt_sb = xpool.tile([128, DK * 512], BF16, tag="xt")
                for k in range(DK):
                    nc.sync.dma_start(
                        xt_sb[:, k * 512:k * 512 + gwid],
                        xt_d[k * 128:(k + 1) * 128, goff:goff + gwid],
                    )

            # ---- phase A: h^T[m] = silu(G^T x) * (U^T x), inter on partitions
            h_sb = hpool.tile([128, IM * 512], BF16, tag="h")
            for m in range(IM):
                blk, moff = m // 4, (m % 4) * 128
                pg = pspool.tile([128, gwid], F32, tag="ps")
                pu = pspool.tile([128, gwid], F32, tag="ps")
                for k in range(DK):
                    nc.tensor.matmul(
                        pg[:],
                        gw_blks[blk][:, k * 512 + moff:k * 512 + moff + 128],
                        xt_sb[:, k * 512:k * 512 + gwid],
                        start=(k == 0),
                        stop=(k == DK - 1),
                    )
                for k in range(DK):
                    nc.tensor.matmul(
                        pu[:],
                        uw_blks[blk][:, k * 512 + moff:k * 512 + moff + 128],
                        xt_sb[:, k * 512:k * 512 + gwid],
                        start=(k == 0),
                        stop=(k == DK - 1),
                    )
                s_sb = spool.tile([128, gwid], F32, tag="s")
                nc.scalar.activation(
                    s_sb[:], pg[:], mybir.ActivationFunctionType.Silu
                )
                nc.vector.tensor_mul(
                    h_sb[:, m * 512:m * 512 + gwid], s_sb[:], pu[:]
                )

            # ---- phase B: y[tok] = h^T.T @ Dw, scaled by combine weight ----
            # m outermost so each streamed dw tile is consumed and released;
            # all (half, t) accumulators live across the m loop.
            n_t = gwid // 128
            py_tiles = [
                [
                    pspool.tile([128, 512], F32, tag="ps", name=f"py_{half}_{t}")
                    for t in range(n_t)
                ]
                for half in range(2)
            ]
            for m in range(IM):
                dwt = dwpool.tile([128, HIDDEN], BF16, tag="dw")
                nc.sync.dma_start(dwt[:], dw_d[m * 128:(m + 1) * 128, :])
                for half in range(2):
                    for t in range(n_t):
                        nc.tensor.matmul(
                            py_tiles[half][t][:],
                            h_sb[:, m * 512 + t * 128:m * 512 + (t + 1) * 128],
                            dwt[:, half * 512:(half + 1) * 512],
                            start=(m == 0),
                            stop=(m == IM - 1),
                        )
            for t in range(n_t):
                out_sb = opool.tile([128, HIDDEN], F32, tag="out")
                tile_idx = goff // 128 + t
                for half in range(2):
                    nc.vector.tensor_scalar_mul(
                        out_sb[:, half * 512:(half + 1) * 512],
                        py_tiles[half][t][:],
                        cw_sb[:, tile_idx:tile_idx + 1],
                    )
                nc.sync.dma_start(
                    y_d[tile_idx * 128:(tile_idx + 1) * 128, :], out_sb[:]
                )

    _split_multi_waits(nc)
    return nc


@functools.lru_cache(maxsize=2)
def _compiled_ncs(c_tokens):
    return build_expert_kernel(c_tokens)


def _route_host(x, router_w):
    """fp32 host routing identical to the jax reference."""
    logits = x @ router_w                                     # [T, E]
    m = logits.max(axis=-1, keepdims=True)
    p = np.exp(logits - m)
    p /= p.sum(axis=-1, keepdims=True)
    idx = np.argsort(-p, axis=-1, kind="stable")[:, :TOP_K]   # [T, K]
    w = np.take_along_axis(p, idx, axis=-1)
    w = w / w.sum(axis=-1, keepdims=True)
    return p, idx, w


def kernel(hidden_states, router_w, gate_w, up_w, down_w):
    hidden_states = np.asarray(hidden_states, dtype=np.float32)
    router_w = np.asarray(router_w, dtype=np.float32)
    x = hidden_states.reshape(T, HIDDEN)

    probs, idx, w = _route_host(x, router_w)

    # load-balance loss (host, fp32, matches reference formula)
    counts = np.bincount(idx.ravel(), minlength=NUM_EXPERTS)
    expert_usage = counts.astype(np.float32) / np.float32(T)
    mean_probs = probs.mean(axis=0, dtype=np.float32)
    loss = np.float32((expert_usage * mean_probs).sum() * NUM_EXPERTS)

    # capacity: padded max expert load (deterministic inputs -> 2304)
    c_tokens = max(2304, int(-(-counts.max() // 128) * 128))

    xb = x.astype(ml_dtypes.bfloat16)
    # pre-block gate/up to [E, INTER//512, HIDDEN, 512] -> rows contiguous
    gwb = (
        np.asarray(gate_w)
        .astype(ml_dtypes.bfloat16)
        .reshape(NUM_EXPERTS, HIDDEN, INTER // 512, 512)
        .transpose(0, 2, 1, 3)
        .reshape(NUM_EXPERTS, INTER // 512 * HIDDEN, 512)
    )
    uwb = (
        np.asarray(up_w)
        .astype(ml_dtypes.bfloat16)
        .reshape(NUM_EXPERTS, HIDDEN, INTER // 512, 512)
        .transpose(0, 2, 1, 3)
        .reshape(NUM_EXPERTS, INTER // 512 * HIDDEN, 512)
    )
    dwb = np.asarray(down_w).astype(ml_dtypes.bfloat16)

    in_maps = []
    tok_lists = []
    for e in range(NUM_EXPERTS):
        tok_e, k_e = np.nonzero(idx == e)
        n_e = len(tok_e)
        tok_lists.append((tok_e, n_e))
        xt = np.zeros((HIDDEN, c_tokens), dtype=ml_dtypes.bfloat16)
        xt[:, :n_e] = xb[tok_e].T
        cw = np.zeros((c_tokens, 1), dtype=np.float32)
        cw[:n_e, 0] = w[tok_e, k_e]
        in_maps.append(
            {
                "xt": xt,
                "gw": np.ascontiguousarray(gwb[e]),
                "uw": np.ascontiguousarray(uwb[e]),
                "dw": np.ascontiguousarray(dwb[e]),
                "cw": cw,
            }
        )

    nc = _compiled_ncs(c_tokens)
    res = run_bass_kernel_spmd(nc, in_maps, core_ids=list(range(N_CORES)))
    global last_results
    last_results = res

    out = np.zeros((T, HIDDEN), dtype=np.float32)
    for e in range(NUM_EXPERTS):
        tok_e, n_e = tok_lists[e]
        out[tok_e] += res.results[e]["y"][:n_e]

    return out.reshape(B, S, HIDDEN), loss


# revision 25
# speedup vs baseline: 1.1098x; 1.0326x over previous
"""MoE top-2 routing kernel for 8 TRN2 NeuronCores (expert parallelism).

Strategy: the router (8192x1024 @ 1024x8 + softmax + top-2) is computed on
host in fp32; tokens are dispatched to the core that owns their expert
(expert e -> core e).  Each core runs the expert MLP
    y = (silu(x @ G) * (x @ U)) @ Dw   scaled per-row by the combine weight
over its padded token batch in bf16 (fp32 PSUM accumulation).  Host
scatter-adds the per-expert outputs back into the [B,S,D] output.

Layout per core (SBUF-resident unless noted):
  gw, uw [1024, 4096] bf16    lhsT for the gate/up matmuls (K=D on partitions)
  xt     [1024, C]    bf16    gathered tokens, transposed (streamed per group)
  dw     [4096, 1024] bf16    down weights (streamed per token group)
  h      [4096, gw]   bf16    silu(gate)*up, transposed (inter on partitions)
                              = exactly the lhsT layout the down matmul needs
Token groups of 512 keep every matmul at N=512 (or 256 for the tail group).
"""

import functools
import sys
import types
from contextlib import ExitStack

import numpy as np
import ml_dtypes

import concourse.bass as bass
import concourse.tile as tile
import concourse.mybir as mybir
from concourse.vector_clock import ScopedClock
from concourse.bass_utils import run_bass_kernel_spmd

# ---------------------------------------------------------------------------
# problem constants (hardcoded per contract)
# ---------------------------------------------------------------------------
NUM_EXPERTS = 8
TOP_K = 2
HIDDEN = 1024          # D
INTER = 4096           # I
B, S = 4, 2048
T = B * S              # 8192 tokens
N_CORES = 8

DK = HIDDEN // 128     # 8  k-tiles over D
IM = INTER // 128      # 32 m-tiles over I

BF16 = mybir.dt.bfloat16
F32 = mybir.dt.float32

# BassKernelResults of the most recent device run (for test harnesses)
last_results = None

# walrus in this container rejects >1 sync wait per instruction; Tile's
# kernel-tail drain aggregates one wait per active proc.  Split them across
# preceding same-engine NOPs.
_MAX_WAITS = 1


def _patched_drain_and_barrier(self, tick_clock, wait_clock):
    nc = self.nc
    probe = nc.sync.nop(nofuse=True)
    wait_clock.add_sem_waits(probe.ins, ScopedClock({None: tick_clock.global_clock}))
    si = probe.ins.sync_info
    waits = list(si.on_wait) if si is not None else []
    if si is not None:
        si.on_wait[:] = waits[:_MAX_WAITS]
    rest = waits[_MAX_WAITS:]
    while rest:
        nop = nc.sync.nop(nofuse=True)
        nop.ins.sync_info = mybir.SyncInfo(on_wait=rest[:_MAX_WAITS], on_update=[])
        rest = rest[_MAX_WAITS:]
    nc.sync.drain()
    nc.all_engine_barrier()
    assert self.sems is not None
    popped = nc._tile_sem_poison_stack.pop()
    assert popped is self._sem_poison
    nc.clear_and_free_semaphores(list(self.sems.allocated().values()))
    nc.all_engine_barrier()


tile.TileContext._drain_and_barrier = _patched_drain_and_barrier


def _split_multi_waits(nc):
    """Safety net: hoist extra sync waits from any instruction onto
    preceding same-engine NOPs (walrus here allows 1 wait per inst)."""
    n = 0
    for f in nc.m.functions:
        for bb in f.blocks:
            new = []
            for ins in bb.instructions:
                si = getattr(ins, "sync_info", None)
                if si is not None and len(si.on_wait) > _MAX_WAITS:
                    waits = list(si.on_wait)
                    rest = waits[_MAX_WAITS:]
                    si.on_wait[:] = waits[:_MAX_WAITS]
                    while rest:
                        n += 1
                        nop = mybir.InstNoOp(
                            name=f"waitsplit-{n}",
                            engine=ins.engine,
                            ins=[],
                            outs=[],
                            sync_info=mybir.SyncInfo(
                                on_wait=rest[:_MAX_WAITS], on_update=[]
                            ),
                        )
                        new.append(nop)
                        rest = rest[_MAX_WAITS:]
                new.append(ins)
            if n:
                bb.instructions[:] = new
    return n


def _token_groups(c):
    groups = []
    off = 0
    while off < c:
        w = min(512, c - off)
        groups.append((off, w))
        off += w
    return groups


def build_expert_kernel(c_tokens, secondary=False):
    """One expert's MLP over c_tokens padded tokens (SPMD across 8 cores).

    With secondary=True, a 128-token segment for a second expert (own
    weight inputs gw2/uw2/dw2, tokens xt2) is appended; its SBUF space
    reuses the main weight slots after the last main group releases them.
    """
    nc = bass.Bass()
    c_out = c_tokens + (128 if secondary else 0)
    xt_d = nc.dram_tensor("xt", [HIDDEN, c_tokens], BF16, kind="ExternalInput")
    # gate/up weights pre-blocked host-side into [INTER//512, HIDDEN, 512]
    # so each (block, k) DMA reads contiguous rows
    gw_d = nc.dram_tensor("gw", [INTER // 512 * HIDDEN, 512], BF16, kind="ExternalInput")
    uw_d = nc.dram_tensor("uw", [INTER // 512 * HIDDEN, 512], BF16, kind="ExternalInput")
    dw_d = nc.dram_tensor("dw", [INTER, HIDDEN], BF16, kind="ExternalInput")
    cw_d = nc.dram_tensor("cw", [c_out, 1], F32, kind="ExternalInput")
    if secondary:
        xt2_d = nc.dram_tensor("xt2", [HIDDEN, 128], BF16, kind="ExternalInput")
        gw2_d = nc.dram_tensor(
            "gw2", [INTER // 512 * HIDDEN, 512], BF16, kind="ExternalInput"
        )
        uw2_d = nc.dram_tensor(
            "uw2", [INTER // 512 * HIDDEN, 512], BF16, kind="ExternalInput"
        )
        dw2_d = nc.dram_tensor("dw2", [INTER, HIDDEN], BF16, kind="ExternalInput")
    y_d = nc.dram_tensor("y", [c_out, HIDDEN], F32, kind="ExternalOutput")

    groups = _token_groups(c_tokens)
    n_tok_tiles = c_out // 128

    with ExitStack() as ctx:
        tc = ctx.enter_context(tile.TileContext(nc))
        wpool = ctx.enter_context(tc.tile_pool(name="weights", bufs=1))
        dwpool = ctx.enter_context(tc.tile_pool(name="dw", bufs=8))
        xpool = ctx.enter_context(tc.tile_pool(name="xt", bufs=2))
        hpool = ctx.enter_context(tc.tile_pool(name="h", bufs=1))
        spool = ctx.enter_context(tc.tile_pool(name="s", bufs=2))
        opool = ctx.enter_context(tc.tile_pool(name="out", bufs=2))
        # one shared 8-slot PSUM pool: phase A cycles gate/up pairs through
        # it (2 live), phase B holds all 8 down-accumulators at once
        pspool = ctx.enter_context(tc.tile_pool(name="ps", bufs=8, space="PSUM"))

        # resident gate/up weights, split into 512-column blocks (separate
        # tiles) so phase A's first m-tiles only depend on block 0's DMA.
        # block b tile layout: k-tile k at col k*512, covering inter columns
        # [b*512, (b+1)*512) of the original [HIDDEN, INTER] weight.
        n_blk = INTER // 512  # 8
        gw_blks = [
            wpool.tile([128, DK * 512], BF16, tag=f"gw{b}", name=f"gwb{b}")
            for b in range(n_blk)
        ]
        uw_blks = [
            wpool.tile([128, DK * 512], BF16, tag=f"uw{b}", name=f"uwb{b}")
            for b in range(n_blk)
        ]

        def load_wblock(b):
            for k in range(DK):
                r0 = b * HIDDEN + k * 128
                nc.sync.dma_start(
                    gw_blks[b][:, k * 512:(k + 1) * 512], gw_d[r0:r0 + 128, :]
                )
                nc.sync.dma_start(
                    uw_blks[b][:, k * 512:(k + 1) * 512], uw_d[r0:r0 + 128, :]
                )

        # group 0's xt interleaved with gate block 0 so the first
        # accumulation group can start after a couple of DMAs
        xt_first = xpool.tile([128, DK * 512], BF16, tag="xt", name="xt_first")
        g0off, g0wid = groups[0]
        for k in range(DK):
            nc.sync.dma_start(
                xt_first[:, k * 512:k * 512 + g0wid],
                xt_d[k * 128:(k + 1) * 128, g0off:g0off + g0wid],
            )
            r0 = k * 128
            nc.sync.dma_start(
                gw_blks[0][:, k * 512:(k + 1) * 512], gw_d[r0:r0 + 128, :]
            )
            nc.sync.dma_start(
                uw_blks[0][:, k * 512:(k + 1) * 512], uw_d[r0:r0 + 128, :]
            )
        for b in range(1, n_blk):
            load_wblock(b)
        # combine weights: column t = token tile t, [128, 1] each (tiny)
        cw_sb = wpool.tile([128, n_tok_tiles], F32, tag="cw")
        for t in range(n_tok_tiles):
            nc.sync.dma_start(cw_sb[:, t:t + 1], cw_d[t * 128:(t + 1) * 128, :])

        for gi, (goff, gwid) in enumerate(groups):
            # ---- load x^T for this token group: k-tile k at col k*512 ----
            if gi == 0:
                xt_sb = xt_first
            else:
                xt_sb = xpool.tile([128, DK * 512], BF16, tag="xt")
                for k in range(DK):
                    nc.sync.dma_start(
                        xt_sb[:, k * 512:k * 512 + gwid],
                        xt_d[k * 128:(k + 1) * 128, goff:goff + gwid],
                    )

            # ---- phase A: h^T[m] = silu(G^T x) * (U^T x), inter on partitions
            h_sb = hpool.tile([128, IM * 512], BF16, tag="h")
            for m in range(IM):
                blk, moff = m // 4, (m % 4) * 128
                pg = pspool.tile([128, gwid], F32, tag="ps")
                pu = pspool.tile([128, gwid], F32, tag="ps")
                for k in range(DK):
                    nc.tensor.matmul(
                        pg[:],
                        gw_blks[blk][:, k * 512 + moff:k * 512 + moff + 128],
                        xt_sb[:, k * 512:k * 512 + gwid],
                        start=(k == 0),
                        stop=(k == DK - 1),
                    )
                for k in range(DK):
                    nc.tensor.matmul(
                        pu[:],
                        uw_blks[blk][:, k * 512 + moff:k * 512 + moff + 128],
                        xt_sb[:, k * 512:k * 512 + gwid],
                        start=(k == 0),
                        stop=(k == DK - 1),
                    )
                s_sb = spool.tile([128, gwid], F32, tag="s")
                nc.scalar.activation(
                    s_sb[:], pg[:], mybir.ActivationFunctionType.Silu
                )
                nc.vector.tensor_mul(
                    h_sb[:, m * 512:m * 512 + gwid], s_sb[:], pu[:]
                )

            # ---- phase B: y[tok] = h^T.T @ Dw, scaled by combine weight ----
            # m outermost so each streamed dw tile is consumed and released;
            # all (half, t) accumulators live across the m loop.
            n_t = gwid // 128
            py_tiles = [
                [
                    pspool.tile([128, 512], F32, tag="ps", name=f"py_{half}_{t}")
                    for t in range(n_t)
                ]
                for half in range(2)
            ]
            for m in range(IM):
                dwt = dwpool.tile([128, HIDDEN], BF16, tag="dw")
                nc.sync.dma_start(dwt[:], dw_d[m * 128:(m + 1) * 128, :])
                for half in range(2):
                    for t in range(n_t):
                        nc.tensor.matmul(
                            py_tiles[half][t][:],
                            h_sb[:, m * 512 + t * 128:m * 512 + (t + 1) * 128],
                            dwt[:, half * 512:(half + 1) * 512],
                            start=(m == 0),
                            stop=(m == IM - 1),
                        )
            for t in range(n_t):
                out_sb = opool.tile([128, HIDDEN], F32, tag="out")
                tile_idx = goff // 128 + t
                for half in range(2):
                    nc.vector.tensor_scalar_mul(
                        out_sb[:, half * 512:(half + 1) * 512],
                        py_tiles[half][t][:],
                        cw_sb[:, tile_idx:tile_idx + 1],
                    )
                nc.sync.dma_start(
                    y_d[tile_idx * 128:(tile_idx + 1) * 128, :], out_sb[:]
                )

        if secondary:
            # ---- secondary expert: one 128-token tile.  Weight blocks are
            # allocated into the same tags as the main weights, so their
            # DMAs stream in as the last main group releases each slot.
            xt2_sb = wpool.tile([128, DK * 128], BF16, tag="xt2")
            for k in range(DK):
                nc.sync.dma_start(
                    xt2_sb[:, k * 128:(k + 1) * 128], xt2_d[k * 128:(k + 1) * 128, :]
                )
            gw2_blks = [
                wpool.tile([128, DK * 512], BF16, tag=f"gw{b}", name=f"gw2b{b}")
                for b in range(n_blk)
            ]
            uw2_blks = [
                wpool.tile([128, DK * 512], BF16, tag=f"uw{b}", name=f"uw2b{b}")
                for b in range(n_blk)
            ]
            for b in range(n_blk):
                for k in range(DK):
                    r0 = b * HIDDEN + k * 128
                    nc.sync.dma_start(
                        gw2_blks[b][:, k * 512:(k + 1) * 512], gw2_d[r0:r0 + 128, :]
                    )
                    nc.sync.dma_start(
                        uw2_blks[b][:, k * 512:(k + 1) * 512], uw2_d[r0:r0 + 128, :]
                    )

            h2_sb = hpool.tile([128, IM * 512], BF16, tag="h", name="h2")
            for m in range(IM):
                blk, moff = m // 4, (m % 4) * 128
                pg = pspool.tile([128, 128], F32, tag="ps", name="pg2")
                pu = pspool.tile([128, 128], F32, tag="ps", name="pu2")
                for k in range(DK):
                    nc.tensor.matmul(
                        pg[:],
                        gw2_blks[blk][:, k * 512 + moff:k * 512 + moff + 128],
                        xt2_sb[:, k * 128:(k + 1) * 128],
                        start=(k == 0),
                        stop=(k == DK - 1),
                    )
                for k in range(DK):
                    nc.tensor.matmul(
                        pu[:],
                        uw2_blks[blk][:, k * 512 + moff:k * 512 + moff + 128],
                        xt2_sb[:, k * 128:(k + 1) * 128],
                        start=(k == 0),
                        stop=(k == DK - 1),
                    )
                s2_sb = spool.tile([128, 128], F32, tag="s", name="s2")
                nc.scalar.activation(
                    s2_sb[:], pg[:], mybir.ActivationFunctionType.Silu
                )
                nc.vector.tensor_mul(
                    h2_sb[:, m * 512:m * 512 + 128], s2_sb[:], pu[:]
                )

            py2 = [
                pspool.tile([128, 512], F32, tag="ps", name=f"py2_{half}")
                for half in range(2)
            ]
            for m in range(IM):
                dwt = dwpool.tile([128, HIDDEN], BF16, tag="dw", name="dw2t")
                nc.sync.dma_start(dwt[:], dw2_d[m * 128:(m + 1) * 128, :])
                for half in range(2):
                    nc.tensor.matmul(
                        py2[half][:],
                        h2_sb[:, m * 512:m * 512 + 128],
                        dwt[:, half * 512:(half + 1) * 512],
                        start=(m == 0),
                        stop=(m == IM - 1),
                    )
            t2 = c_tokens // 128
            out2_sb = opool.tile([128, HIDDEN], F32, tag="out", name="out2")
            for half in range(2):
                nc.vector.tensor_scalar_mul(
                    out2_sb[:, half * 512:(half + 1) * 512],
                    py2[half][:],
                    cw_sb[:, t2:t2 + 1],
                )
            nc.sync.dma_start(y_d[c_tokens:c_tokens + 128, :], out2_sb[:])

    _split_multi_waits(nc)
    return nc


@functools.lru_cache(maxsize=2)
def _compiled_ncs(c_tokens, secondary=False):
    return build_expert_kernel(c_tokens, secondary)


def _route_host(x, router_w):
    """fp32 host routing identical to the jax reference."""
    logits = x @ router_w                                     # [T, E]
    m = logits.max(axis=-1, keepdims=True)
    p = np.exp(logits - m)
    p /= p.sum(axis=-1, keepdims=True)
    idx = np.argsort(-p, axis=-1, kind="stable")[:, :TOP_K]   # [T, K]
    w = np.take_along_axis(p, idx, axis=-1)
    w = w / w.sum(axis=-1, keepdims=True)
    return p, idx, w


def kernel(hidden_states, router_w, gate_w, up_w, down_w):
    global last_results
    hidden_states = np.asarray(hidden_states, dtype=np.float32)
    router_w = np.asarray(router_w, dtype=np.float32)
    x = hidden_states.reshape(T, HIDDEN)

    probs, idx, w = _route_host(x, router_w)

    # load-balance loss (host, fp32, matches reference formula)
    counts = np.bincount(idx.ravel(), minlength=NUM_EXPERTS)
    expert_usage = counts.astype(np.float32) / np.float32(T)
    mean_probs = probs.mean(axis=0, dtype=np.float32)
    loss = np.float32((expert_usage * mean_probs).sum() * NUM_EXPERTS)

    # capacity: padded max expert load (deterministic inputs -> 2304)
    c_tokens = max(2304, int(-(-counts.max() // 128) * 128))

    xb = x.astype(ml_dtypes.bfloat16)
    # pre-block gate/up to [E, INTER//512, HIDDEN, 512] -> rows contiguous
    gwb = (
        np.asarray(gate_w)
        .astype(ml_dtypes.bfloat16)
        .reshape(NUM_EXPERTS, HIDDEN, INTER // 512, 512)
        .transpose(0, 2, 1, 3)
        .reshape(NUM_EXPERTS, INTER // 512 * HIDDEN, 512)
    )
    uwb = (
        np.asarray(up_w)
        .astype(ml_dtypes.bfloat16)
        .reshape(NUM_EXPERTS, HIDDEN, INTER // 512, 512)
        .transpose(0, 2, 1, 3)
        .reshape(NUM_EXPERTS, INTER // 512 * HIDDEN, 512)
    )
    dwb = np.asarray(down_w).astype(ml_dtypes.bfloat16)

    routed = []
    for e in range(NUM_EXPERTS):
        tok_e, k_e = np.nonzero(idx == e)
        routed.append((tok_e, k_e))

    # ---- balanced plan: 2048 main tokens/core + one <=128-token secondary
    # tile carrying another expert's overflow (with replicated weights) ----
    T1 = 2048
    chunks = []
    for e in range(NUM_EXPERTS):
        tok_e, k_e = routed[e]
        for i in range(T1, len(tok_e), 128):
            chunks.append((e, tok_e[i:i + 128], k_e[i:i + 128]))
    balanced = len(chunks) <= NUM_EXPERTS

    if balanced:
        c_main = T1
        in_maps = []
        for e in range(NUM_EXPERTS):
            tok_e, k_e = routed[e]
            n_main = min(len(tok_e), T1)
            xt = np.zeros((HIDDEN, c_main), dtype=ml_dtypes.bfloat16)
            xt[:, :n_main] = xb[tok_e[:n_main]].T
            cw = np.zeros((c_main + 128, 1), dtype=np.float32)
            cw[:n_main, 0] = w[tok_e[:n_main], k_e[:n_main]]
            m = {
                "xt": xt,
                "gw": np.ascontiguousarray(gwb[e]),
                "uw": np.ascontiguousarray(uwb[e]),
                "dw": np.ascontiguousarray(dwb[e]),
                "cw": cw,
            }
            if e < len(chunks):
                e2, tok2, k2 = chunks[e]
                n2 = len(tok2)
                xt2 = np.zeros((HIDDEN, 128), dtype=ml_dtypes.bfloat16)
                xt2[:, :n2] = xb[tok2].T
                cw[c_main:c_main + n2, 0] = w[tok2, k2]
                m["xt2"] = xt2
                m["gw2"] = np.ascontiguousarray(gwb[e2])
                m["uw2"] = np.ascontiguousarray(uwb[e2])
                m["dw2"] = np.ascontiguousarray(dwb[e2])
            else:
                zw = np.zeros((INTER // 512 * HIDDEN, 512), dtype=ml_dtypes.bfloat16)
                m["xt2"] = np.zeros((HIDDEN, 128), dtype=ml_dtypes.bfloat16)
                m["gw2"] = zw
                m["uw2"] = zw
                m["dw2"] = np.zeros((INTER, HIDDEN), dtype=ml_dtypes.bfloat16)
            in_maps.append(m)

        nc = _compiled_ncs(c_main, True)
        res = run_bass_kernel_spmd(nc, in_maps, core_ids=list(range(N_CORES)))
        last_results = res

        out = np.zeros((T, HIDDEN), dtype=np.float32)
        for e in range(NUM_EXPERTS):
            tok_e, k_e = routed[e]
            n_main = min(len(tok_e), T1)
            out[tok_e[:n_main]] += res.results[e]["y"][:n_main]
            if e < len(chunks):
                _, tok2, _ = chunks[e]
                out[tok2] += res.results[e]["y"][c_main:c_main + len(tok2)]
        return out.reshape(B, S, HIDDEN), loss

    # ---- fallback: pure per-expert padding ----
    in_maps = []
    for e in range(NUM_EXPERTS):
        tok_e, k_e = routed[e]
        n_e = len(tok_e)
        xt = np.zeros((HIDDEN, c_tokens), dtype=ml_dtypes.bfloat16)
        xt[:, :n_e] = xb[tok_e].T
        cw = np.zeros((c_tokens, 1), dtype=np.float32)
        cw[:n_e, 0] = w[tok_e, k_e]
        in_maps.append(
            {
                "xt": xt,
                "gw": np.ascontiguousarray(gwb[e]),
                "uw": np.ascontiguousarray(uwb[e]),
                "dw": np.ascontiguousarray(dwb[e]),
                "cw": cw,
            }
        )

    nc = _compiled_ncs(c_tokens)
    res = run_bass_kernel_spmd(nc, in_maps, core_ids=list(range(N_CORES)))
    last_results = res

    out = np.zeros((T, HIDDEN), dtype=np.float32)
    for e in range(NUM_EXPERTS):
        tok_e, _ = routed[e]
        out[tok_e] += res.results[e]["y"][:len(tok_e)]

    return out.reshape(B, S, HIDDEN), loss


# revision 30
# speedup vs baseline: 1.1372x; 1.0247x over previous
"""MoE top-2 routing kernel for 8 TRN2 NeuronCores (expert parallelism).

Strategy: the router (8192x1024 @ 1024x8 + softmax + top-2) is computed on
host in fp32; tokens are dispatched to the core that owns their expert
(expert e -> core e).  Each core runs the expert MLP
    y = (silu(x @ G) * (x @ U)) @ Dw   scaled per-row by the combine weight
over its padded token batch in bf16 (fp32 PSUM accumulation).  Host
scatter-adds the per-expert outputs back into the [B,S,D] output.

Layout per core (SBUF-resident unless noted):
  gw, uw [1024, 4096] bf16    lhsT for the gate/up matmuls (K=D on partitions)
  xt     [1024, C]    bf16    gathered tokens, transposed (streamed per group)
  dw     [4096, 1024] bf16    down weights (streamed per token group)
  h      [4096, gw]   bf16    silu(gate)*up, transposed (inter on partitions)
                              = exactly the lhsT layout the down matmul needs
Token groups of 512 keep every matmul at N=512 (or 256 for the tail group).
"""

import functools
from contextlib import ExitStack

import numpy as np
import ml_dtypes

import concourse.bass as bass
import concourse.tile as tile
import concourse.mybir as mybir
from concourse.vector_clock import ScopedClock
from concourse.bass_utils import run_bass_kernel_spmd

# ---------------------------------------------------------------------------
# problem constants (hardcoded per contract)
# ---------------------------------------------------------------------------
NUM_EXPERTS = 8
TOP_K = 2
HIDDEN = 1024          # D
INTER = 4096           # I
B, S = 4, 2048
T = B * S              # 8192 tokens
N_CORES = 8

DK = HIDDEN // 128     # 8  k-tiles over D
IM = INTER // 128      # 32 m-tiles over I

BF16 = mybir.dt.bfloat16
F32 = mybir.dt.float32

# BassKernelResults of the most recent device run (for test harnesses)
last_results = None

# walrus in this container rejects >1 sync wait per instruction; Tile's
# kernel-tail drain aggregates one wait per active proc.  Split them across
# preceding same-engine NOPs.
_MAX_WAITS = 1


def _patched_drain_and_barrier(self, tick_clock, wait_clock):
    nc = self.nc
    probe = nc.sync.nop(nofuse=True)
    wait_clock.add_sem_waits(probe.ins, ScopedClock({None: tick_clock.global_clock}))
    si = probe.ins.sync_info
    waits = list(si.on_wait) if si is not None else []
    if si is not None:
        si.on_wait[:] = waits[:_MAX_WAITS]
    rest = waits[_MAX_WAITS:]
    while rest:
        nop = nc.sync.nop(nofuse=True)
        nop.ins.sync_info = mybir.SyncInfo(on_wait=rest[:_MAX_WAITS], on_update=[])
        rest = rest[_MAX_WAITS:]
    nc.sync.drain()
    nc.all_engine_barrier()
    assert self.sems is not None
    popped = nc._tile_sem_poison_stack.pop()
    assert popped is self._sem_poison
    nc.clear_and_free_semaphores(list(self.sems.allocated().values()))
    nc.all_engine_barrier()


tile.TileContext._drain_and_barrier = _patched_drain_and_barrier


def _split_multi_waits(nc):
    """Safety net: hoist extra sync waits from any instruction onto
    preceding same-engine NOPs (walrus here allows 1 wait per inst)."""
    n = 0
    for f in nc.m.functions:
        for bb in f.blocks:
            new = []
            for ins in bb.instructions:
                si = getattr(ins, "sync_info", None)
                if si is not None and len(si.on_wait) > _MAX_WAITS:
                    waits = list(si.on_wait)
                    rest = waits[_MAX_WAITS:]
                    si.on_wait[:] = waits[:_MAX_WAITS]
                    while rest:
                        n += 1
                        nop = mybir.InstNoOp(
                            name=f"waitsplit-{n}",
                            engine=ins.engine,
                            ins=[],
                            outs=[],
                            sync_info=mybir.SyncInfo(
                                on_wait=rest[:_MAX_WAITS], on_update=[]
                            ),
                        )
                        new.append(nop)
                        rest = rest[_MAX_WAITS:]
                new.append(ins)
            if n:
                bb.instructions[:] = new
    return n


def _token_groups(c):
    groups = []
    off = 0
    while off < c:
        w = min(512, c - off)
        groups.append((off, w))
        off += w
    return groups


def build_expert_kernel(c_tokens, secondary=False):
    """One expert's MLP over c_tokens padded tokens (SPMD across 8 cores).

    With secondary=True, a 128-token segment for a second expert (own
    weight inputs gw2/uw2/dw2, tokens xt2) is appended; its SBUF space
    reuses the main weight slots after the last main group releases them.
    """
    nc = bass.Bass()
    c_out = c_tokens + (128 if secondary else 0)
    xt_d = nc.dram_tensor("xt", [HIDDEN, c_tokens], BF16, kind="ExternalInput")
    # gate/up weights pre-blocked host-side into [INTER//512, HIDDEN, 512]
    # so each (block, k) DMA reads contiguous rows
    gw_d = nc.dram_tensor("gw", [INTER // 512 * HIDDEN, 512], BF16, kind="ExternalInput")
    uw_d = nc.dram_tensor("uw", [INTER // 512 * HIDDEN, 512], BF16, kind="ExternalInput")
    dw_d = nc.dram_tensor("dw", [INTER, HIDDEN], BF16, kind="ExternalInput")
    cw_d = nc.dram_tensor("cw", [c_out, 1], F32, kind="ExternalInput")
    if secondary:
        xt2_d = nc.dram_tensor("xt2", [HIDDEN, 128], BF16, kind="ExternalInput")
        gw2_d = nc.dram_tensor(
            "gw2", [INTER // 512 * HIDDEN, 512], BF16, kind="ExternalInput"
        )
        uw2_d = nc.dram_tensor(
            "uw2", [INTER // 512 * HIDDEN, 512], BF16, kind="ExternalInput"
        )
        dw2_d = nc.dram_tensor("dw2", [INTER, HIDDEN], BF16, kind="ExternalInput")
    y_d = nc.dram_tensor("y", [c_out, HIDDEN], F32, kind="ExternalOutput")

    groups = _token_groups(c_tokens)
    n_tok_tiles = c_out // 128

    with ExitStack() as ctx:
        tc = ctx.enter_context(tile.TileContext(nc))
        wpool = ctx.enter_context(tc.tile_pool(name="weights", bufs=1))
        dwpool = ctx.enter_context(tc.tile_pool(name="dw", bufs=8))
        xpool = ctx.enter_context(tc.tile_pool(name="xt", bufs=2))
        hpool = ctx.enter_context(tc.tile_pool(name="h", bufs=1))
        spool = ctx.enter_context(tc.tile_pool(name="s", bufs=2))
        opool = ctx.enter_context(tc.tile_pool(name="out", bufs=2))
        # one shared 8-slot PSUM pool: phase A cycles gate/up pairs through
        # it (2 live), phase B holds all 8 down-accumulators at once
        pspool = ctx.enter_context(tc.tile_pool(name="ps", bufs=8, space="PSUM"))

        # resident gate/up weights, split into 512-column blocks (separate
        # tiles) so phase A's first m-tiles only depend on block 0's DMA.
        # block b tile layout: k-tile k at col k*512, covering inter columns
        # [b*512, (b+1)*512) of the original [HIDDEN, INTER] weight.
        n_blk = INTER // 512  # 8
        gw_blks = [
            wpool.tile([128, DK * 512], BF16, tag=f"gw{b}", name=f"gwb{b}")
            for b in range(n_blk)
        ]
        uw_blks = [
            wpool.tile([128, DK * 512], BF16, tag=f"uw{b}", name=f"uwb{b}")
            for b in range(n_blk)
        ]

        def load_wblock(b):
            for k in range(DK):
                r0 = b * HIDDEN + k * 128
                nc.sync.dma_start(
                    gw_blks[b][:, k * 512:(k + 1) * 512], gw_d[r0:r0 + 128, :]
                )
                nc.sync.dma_start(
                    uw_blks[b][:, k * 512:(k + 1) * 512], uw_d[r0:r0 + 128, :]
                )

        # group 0's xt interleaved with gate block 0 so the first
        # accumulation group can start after a couple of DMAs
        xt_first = xpool.tile([128, DK * 512], BF16, tag="xt", name="xt_first")
        g0off, g0wid = groups[0]
        for k in range(DK):
            nc.sync.dma_start(
                xt_first[:, k * 512:k * 512 + g0wid],
                xt_d[k * 128:(k + 1) * 128, g0off:g0off + g0wid],
            )
            r0 = k * 128
            nc.sync.dma_start(
                gw_blks[0][:, k * 512:(k + 1) * 512], gw_d[r0:r0 + 128, :]
            )
            nc.sync.dma_start(
                uw_blks[0][:, k * 512:(k + 1) * 512], uw_d[r0:r0 + 128, :]
            )
        for b in range(1, n_blk):
            load_wblock(b)
        # combine weights: column t = token tile t, [128, 1] each (tiny)
        cw_sb = wpool.tile([128, n_tok_tiles], F32, tag="cw")
        for t in range(n_tok_tiles):
            nc.sync.dma_start(cw_sb[:, t:t + 1], cw_d[t * 128:(t + 1) * 128, :])

        for gi, (goff, gwid) in enumerate(groups):
            # ---- load x^T for this token group: k-tile k at col k*512 ----
            if gi == 0:
                xt_sb = xt_first
            else:
                xt_sb = xpool.tile([128, DK * 512], BF16, tag="xt")
                for k in range(DK):
                    nc.sync.dma_start(
                        xt_sb[:, k * 512:k * 512 + gwid],
                        xt_d[k * 128:(k + 1) * 128, goff:goff + gwid],
                    )

            # ---- phase A: h^T[m] = silu(G^T x) * (U^T x), inter on partitions
            h_sb = hpool.tile([128, IM * 512], BF16, tag="h")
            for m in range(IM):
                blk, moff = m // 4, (m % 4) * 128
                pg = pspool.tile([128, gwid], F32, tag="ps")
                pu = pspool.tile([128, gwid], F32, tag="ps")
                for k in range(DK):
                    nc.tensor.matmul(
                        pg[:],
                        gw_blks[blk][:, k * 512 + moff:k * 512 + moff + 128],
                        xt_sb[:, k * 512:k * 512 + gwid],
                        start=(k == 0),
                        stop=(k == DK - 1),
                    )
                for k in range(DK):
                    nc.tensor.matmul(
                        pu[:],
                        uw_blks[blk][:, k * 512 + moff:k * 512 + moff + 128],
                        xt_sb[:, k * 512:k * 512 + gwid],
                        start=(k == 0),
                        stop=(k == DK - 1),
                    )
                s_sb = spool.tile([128, gwid], F32, tag="s")
                nc.scalar.activation(
                    s_sb[:], pg[:], mybir.ActivationFunctionType.Silu
                )
                nc.vector.tensor_mul(
                    h_sb[:, m * 512:m * 512 + gwid], s_sb[:], pu[:]
                )

            # ---- phase B: y[tok] = h^T.T @ Dw, scaled by combine weight ----
            # m outermost so each streamed dw tile is consumed and released;
            # all (half, t) accumulators live across the m loop.
            n_t = gwid // 128
            py_tiles = [
                [
                    pspool.tile([128, 512], F32, tag="ps", name=f"py_{half}_{t}")
                    for t in range(n_t)
                ]
                for half in range(2)
            ]
            for m in range(IM):
                dwt = dwpool.tile([128, HIDDEN], BF16, tag="dw")
                nc.sync.dma_start(dwt[:], dw_d[m * 128:(m + 1) * 128, :])
                for half in range(2):
                    for t in range(n_t):
                        nc.tensor.matmul(
                            py_tiles[half][t][:],
                            h_sb[:, m * 512 + t * 128:m * 512 + (t + 1) * 128],
                            dwt[:, half * 512:(half + 1) * 512],
                            start=(m == 0),
                            stop=(m == IM - 1),
                        )
            for t in range(n_t):
                out_sb = opool.tile([128, HIDDEN], F32, tag="out")
                tile_idx = goff // 128 + t
                for half in range(2):
                    nc.vector.tensor_scalar_mul(
                        out_sb[:, half * 512:(half + 1) * 512],
                        py_tiles[half][t][:],
                        cw_sb[:, tile_idx:tile_idx + 1],
                    )
                nc.sync.dma_start(
                    y_d[tile_idx * 128:(tile_idx + 1) * 128, :], out_sb[:]
                )

        if secondary:
            # ---- secondary expert: one 128-token tile.  Weight blocks are
            # allocated into the same tags as the main weights, so their
            # DMAs stream in as the last main group releases each slot.
            xt2_sb = wpool.tile([128, DK * 128], BF16, tag="xt2")
            for k in range(DK):
                nc.sync.dma_start(
                    xt2_sb[:, k * 128:(k + 1) * 128], xt2_d[k * 128:(k + 1) * 128, :]
                )
            gw2_blks = [
                wpool.tile([128, DK * 512], BF16, tag=f"gw{b}", name=f"gw2b{b}")
                for b in range(n_blk)
            ]
            uw2_blks = [
                wpool.tile([128, DK * 512], BF16, tag=f"uw{b}", name=f"uw2b{b}")
                for b in range(n_blk)
            ]
            for b in range(n_blk):
                for k in range(DK):
                    r0 = b * HIDDEN + k * 128
                    nc.sync.dma_start(
                        gw2_blks[b][:, k * 512:(k + 1) * 512], gw2_d[r0:r0 + 128, :]
                    )
                    nc.sync.dma_start(
                        uw2_blks[b][:, k * 512:(k + 1) * 512], uw2_d[r0:r0 + 128, :]
                    )

            h2_sb = hpool.tile([128, IM * 512], BF16, tag="h", name="h2")
            for m in range(IM):
                blk, moff = m // 4, (m % 4) * 128
                pg = pspool.tile([128, 128], F32, tag="ps", name="pg2")
                pu = pspool.tile([128, 128], F32, tag="ps", name="pu2")
                for k in range(DK):
                    nc.tensor.matmul(
                        pg[:],
                        gw2_blks[blk][:, k * 512 + moff:k * 512 + moff + 128],
                        xt2_sb[:, k * 128:(k + 1) * 128],
                        start=(k == 0),
                        stop=(k == DK - 1),
                    )
                for k in range(DK):
                    nc.tensor.matmul(
                        pu[:],
                        uw2_blks[blk][:, k * 512 + moff:k * 512 + moff + 128],
                        xt2_sb[:, k * 128:(k + 1) * 128],
                        start=(k == 0),
                        stop=(k == DK - 1),
                    )
                s2_sb = spool.tile([128, 128], F32, tag="s", name="s2")
                nc.scalar.activation(
                    s2_sb[:], pg[:], mybir.ActivationFunctionType.Silu
                )
                nc.vector.tensor_mul(
                    h2_sb[:, m * 512:m * 512 + 128], s2_sb[:], pu[:]
                )

            # pre-stage all dw2 m-tiles into the gw2/uw2 block slots (dead
            # after phase A2 reads them) so B2 isn't DMA-bound
            dw2pre = [
                wpool.tile(
                    [128, 4 * HIDDEN],
                    BF16,
                    tag=(f"gw{j}" if j < 4 else f"uw{j - 4}"),
                    name=f"dw2pre{j}",
                )
                for j in range(8)
            ]
            for j in range(8):
                for i in range(4):
                    m = j * 4 + i
                    nc.sync.dma_start(
                        dw2pre[j][:, i * HIDDEN:(i + 1) * HIDDEN],
                        dw2_d[m * 128:(m + 1) * 128, :],
                    )

            py2 = [
                pspool.tile([128, 512], F32, tag="ps", name=f"py2_{half}")
                for half in range(2)
            ]
            for m in range(IM):
                base = (m % 4) * HIDDEN
                halves = [
                    dw2pre[m // 4][:, base + h * 512:base + (h + 1) * 512]
                    for h in range(2)
                ]
                for half in range(2):
                    nc.tensor.matmul(
                        py2[half][:],
                        h2_sb[:, m * 512:m * 512 + 128],
                        halves[half],
                        start=(m == 0),
                        stop=(m == IM - 1),
                    )
            t2 = c_tokens // 128
            out2_sb = opool.tile([128, HIDDEN], F32, tag="out", name="out2")
            for half in range(2):
                nc.vector.tensor_scalar_mul(
                    out2_sb[:, half * 512:(half + 1) * 512],
                    py2[half][:],
                    cw_sb[:, t2:t2 + 1],
                )
            nc.sync.dma_start(y_d[c_tokens:c_tokens + 128, :], out2_sb[:])

    _split_multi_waits(nc)
    return nc


@functools.lru_cache(maxsize=2)
def _compiled_ncs(c_tokens, secondary=False):
    return build_expert_kernel(c_tokens, secondary)


def _route_host(x, router_w):
    """fp32 host routing identical to the jax reference."""
    logits = x @ router_w                                     # [T, E]
    m = logits.max(axis=-1, keepdims=True)
    p = np.exp(logits - m)
    p /= p.sum(axis=-1, keepdims=True)
    idx = np.argsort(-p, axis=-1, kind="stable")[:, :TOP_K]   # [T, K]
    w = np.take_along_axis(p, idx, axis=-1)
    w = w / w.sum(axis=-1, keepdims=True)
    return p, idx, w


def kernel(hidden_states, router_w, gate_w, up_w, down_w):
    global last_results
    hidden_states = np.asarray(hidden_states, dtype=np.float32)
    router_w = np.asarray(router_w, dtype=np.float32)
    x = hidden_states.reshape(T, HIDDEN)

    probs, idx, w = _route_host(x, router_w)

    # load-balance loss (host, fp32, matches reference formula)
    counts = np.bincount(idx.ravel(), minlength=NUM_EXPERTS)
    expert_usage = counts.astype(np.float32) / np.float32(T)
    mean_probs = probs.mean(axis=0, dtype=np.float32)
    loss = np.float32((expert_usage * mean_probs).sum() * NUM_EXPERTS)

    # capacity: padded max expert load (deterministic inputs -> 2304)
    c_tokens = max(2304, int(-(-counts.max() // 128) * 128))

    xb = x.astype(ml_dtypes.bfloat16)
    # pre-block gate/up to [E, INTER//512, HIDDEN, 512] -> rows contiguous
    gwb = (
        np.asarray(gate_w)
        .astype(ml_dtypes.bfloat16)
        .reshape(NUM_EXPERTS, HIDDEN, INTER // 512, 512)
        .transpose(0, 2, 1, 3)
        .reshape(NUM_EXPERTS, INTER // 512 * HIDDEN, 512)
    )
    uwb = (
        np.asarray(up_w)
        .astype(ml_dtypes.bfloat16)
        .reshape(NUM_EXPERTS, HIDDEN, INTER // 512, 512)
        .transpose(0, 2, 1, 3)
        .reshape(NUM_EXPERTS, INTER // 512 * HIDDEN, 512)
    )
    dwb = np.asarray(down_w).astype(ml_dtypes.bfloat16)

    routed = []
    for e in range(NUM_EXPERTS):
        tok_e, k_e = np.nonzero(idx == e)
        routed.append((tok_e, k_e))

    # ---- balanced plan: 2048 main tokens/core + one <=128-token secondary
    # tile carrying another expert's overflow (with replicated weights) ----
    T1 = 2048
    chunks = []
    for e in range(NUM_EXPERTS):
        tok_e, k_e = routed[e]
        for i in range(T1, len(tok_e), 128):
            chunks.append((e, tok_e[i:i + 128], k_e[i:i + 128]))
    balanced = len(chunks) <= NUM_EXPERTS

    if balanced:
        c_main = T1
        in_maps = []
        for e in range(NUM_EXPERTS):
            tok_e, k_e = routed[e]
            n_main = min(len(tok_e), T1)
            xt = np.zeros((HIDDEN, c_main), dtype=ml_dtypes.bfloat16)
            xt[:, :n_main] = xb[tok_e[:n_main]].T
            cw = np.zeros((c_main + 128, 1), dtype=np.float32)
            cw[:n_main, 0] = w[tok_e[:n_main], k_e[:n_main]]
            m = {
                "xt": xt,
                "gw": np.ascontiguousarray(gwb[e]),
                "uw": np.ascontiguousarray(uwb[e]),
                "dw": np.ascontiguousarray(dwb[e]),
                "cw": cw,
            }
            if e < len(chunks):
                e2, tok2, k2 = chunks[e]
                n2 = len(tok2)
                xt2 = np.zeros((HIDDEN, 128), dtype=ml_dtypes.bfloat16)
                xt2[:, :n2] = xb[tok2].T
                cw[c_main:c_main + n2, 0] = w[tok2, k2]
                m["xt2"] = xt2
                m["gw2"] = np.ascontiguousarray(gwb[e2])
                m["uw2"] = np.ascontiguousarray(uwb[e2])
                m["dw2"] = np.ascontiguousarray(dwb[e2])
            else:
                zw = np.zeros((INTER // 512 * HIDDEN, 512), dtype=ml_dtypes.bfloat16)
                m["xt2"] = np.zeros((HIDDEN, 128), dtype=ml_dtypes.bfloat16)
                m["gw2"] = zw
                m["uw2"] = zw
                m["dw2"] = np.zeros((INTER, HIDDEN), dtype=ml_dtypes.bfloat16)
            in_maps.append(m)

        nc = _compiled_ncs(c_main, True)
        res = run_bass_kernel_spmd(nc, in_maps, core_ids=list(range(N_CORES)))
        last_results = res

        out = np.zeros((T, HIDDEN), dtype=np.float32)
        for e in range(NUM_EXPERTS):
            tok_e, k_e = routed[e]
            n_main = min(len(tok_e), T1)
            out[tok_e[:n_main]] += res.results[e]["y"][:n_main]
            if e < len(chunks):
                _, tok2, _ = chunks[e]
                out[tok2] += res.results[e]["y"][c_main:c_main + len(tok2)]
        return out.reshape(B, S, HIDDEN), loss

    # ---- fallback: pure per-expert padding ----
    in_maps = []
    for e in range(NUM_EXPERTS):
        tok_e, k_e = routed[e]
        n_e = len(tok_e)
        xt = np.zeros((HIDDEN, c_tokens), dtype=ml_dtypes.bfloat16)
        xt[:, :n_e] = xb[tok_e].T
        cw = np.zeros((c_tokens, 1), dtype=np.float32)
        cw[:n_e, 0] = w[tok_e, k_e]
        in_maps.append(
            {
                "xt": xt,
                "gw": np.ascontiguousarray(gwb[e]),
                "uw": np.ascontiguousarray(uwb[e]),
                "dw": np.ascontiguousarray(dwb[e]),
                "cw": cw,
            }
        )

    nc = _compiled_ncs(c_tokens)
    res = run_bass_kernel_spmd(nc, in_maps, core_ids=list(range(N_CORES)))
    last_results = res

    out = np.zeros((T, HIDDEN), dtype=np.float32)
    for e in range(NUM_EXPERTS):
        tok_e, _ = routed[e]
        out[tok_e] += res.results[e]["y"][:len(tok_e)]

    return out.reshape(B, S, HIDDEN), loss


# revision 31
# speedup vs baseline: 1.1395x; 1.0021x over previous
"""MoE top-2 routing kernel for 8 TRN2 NeuronCores (expert parallelism).

Strategy: the router (8192x1024 @ 1024x8 + softmax + top-2) is computed on
host in fp32; tokens are dispatched to the core that owns their expert
(expert e -> core e).  Each core runs the expert MLP
    y = (silu(x @ G) * (x @ U)) @ Dw   scaled per-row by the combine weight
over its padded token batch in bf16 (fp32 PSUM accumulation).  Host
scatter-adds the per-expert outputs back into the [B,S,D] output.

Layout per core (SBUF-resident unless noted):
  gw, uw [1024, 4096] bf16    lhsT for the gate/up matmuls (K=D on partitions)
  xt     [1024, C]    bf16    gathered tokens, transposed (streamed per group)
  dw     [4096, 1024] bf16    down weights (streamed per token group)
  h      [4096, gw]   bf16    silu(gate)*up, transposed (inter on partitions)
                              = exactly the lhsT layout the down matmul needs
Token groups of 512 keep every matmul at N=512 (or 256 for the tail group).
"""

import functools
from contextlib import ExitStack

import numpy as np
import ml_dtypes

import concourse.bass as bass
import concourse.tile as tile
import concourse.mybir as mybir
from concourse.vector_clock import ScopedClock
from concourse.bass_utils import run_bass_kernel_spmd

# ---------------------------------------------------------------------------
# problem constants (hardcoded per contract)
# ---------------------------------------------------------------------------
NUM_EXPERTS = 8
TOP_K = 2
HIDDEN = 1024          # D
INTER = 4096           # I
B, S = 4, 2048
T = B * S              # 8192 tokens
N_CORES = 8

DK = HIDDEN // 128     # 8  k-tiles over D
IM = INTER // 128      # 32 m-tiles over I

BF16 = mybir.dt.bfloat16
F32 = mybir.dt.float32

# BassKernelResults of the most recent device run (for test harnesses)
last_results = None

# walrus in this container rejects >1 sync wait per instruction; Tile's
# kernel-tail drain aggregates one wait per active proc.  Split them across
# preceding same-engine NOPs.
_MAX_WAITS = 1


def _patched_drain_and_barrier(self, tick_clock, wait_clock):
    nc = self.nc
    probe = nc.sync.nop(nofuse=True)
    wait_clock.add_sem_waits(probe.ins, ScopedClock({None: tick_clock.global_clock}))
    si = probe.ins.sync_info
    waits = list(si.on_wait) if si is not None else []
    if si is not None:
        si.on_wait[:] = waits[:_MAX_WAITS]
    rest = waits[_MAX_WAITS:]
    while rest:
        nop = nc.sync.nop(nofuse=True)
        nop.ins.sync_info = mybir.SyncInfo(on_wait=rest[:_MAX_WAITS], on_update=[])
        rest = rest[_MAX_WAITS:]
    nc.sync.drain()
    # the split waits above already observed every engine's final semaphore
    # value and the drain flushed the DMA queues, so the sem clears can't
    # race anything — skip the two EVSEM all-engine barriers (~2-3 us each)
    assert self.sems is not None
    popped = nc._tile_sem_poison_stack.pop()
    assert popped is self._sem_poison
    nc.clear_and_free_semaphores(list(self.sems.allocated().values()))


tile.TileContext._drain_and_barrier = _patched_drain_and_barrier


def _split_multi_waits(nc):
    """Safety net: hoist extra sync waits from any instruction onto
    preceding same-engine NOPs (walrus here allows 1 wait per inst)."""
    n = 0
    for f in nc.m.functions:
        for bb in f.blocks:
            new = []
            for ins in bb.instructions:
                si = getattr(ins, "sync_info", None)
                if si is not None and len(si.on_wait) > _MAX_WAITS:
                    waits = list(si.on_wait)
                    rest = waits[_MAX_WAITS:]
                    si.on_wait[:] = waits[:_MAX_WAITS]
                    while rest:
                        n += 1
                        nop = mybir.InstNoOp(
                            name=f"waitsplit-{n}",
                            engine=ins.engine,
                            ins=[],
                            outs=[],
                            sync_info=mybir.SyncInfo(
                                on_wait=rest[:_MAX_WAITS], on_update=[]
                            ),
                        )
                        new.append(nop)
                        rest = rest[_MAX_WAITS:]
                new.append(ins)
            if n:
                bb.instructions[:] = new
    return n


def _token_groups(c):
    groups = []
    off = 0
    while off < c:
        w = min(512, c - off)
        groups.append((off, w))
        off += w
    return groups


def build_expert_kernel(c_tokens, secondary=False):
    """One expert's MLP over c_tokens padded tokens (SPMD across 8 cores).

    With secondary=True, a 128-token segment for a second expert (own
    weight inputs gw2/uw2/dw2, tokens xt2) is appended; its SBUF space
    reuses the main weight slots after the last main group releases them.
    """
    nc = bass.Bass()
    c_out = c_tokens + (128 if secondary else 0)
    xt_d = nc.dram_tensor("xt", [HIDDEN, c_tokens], BF16, kind="ExternalInput")
    # gate/up weights pre-blocked host-side into [INTER//512, HIDDEN, 512]
    # so each (block, k) DMA reads contiguous rows
    gw_d = nc.dram_tensor("gw", [INTER // 512 * HIDDEN, 512], BF16, kind="ExternalInput")
    uw_d = nc.dram_tensor("uw", [INTER // 512 * HIDDEN, 512], BF16, kind="ExternalInput")
    dw_d = nc.dram_tensor("dw", [INTER, HIDDEN], BF16, kind="ExternalInput")
    cw_d = nc.dram_tensor("cw", [c_out, 1], F32, kind="ExternalInput")
    if secondary:
        xt2_d = nc.dram_tensor("xt2", [HIDDEN, 128], BF16, kind="ExternalInput")
        gw2_d = nc.dram_tensor(
            "gw2", [INTER // 512 * HIDDEN, 512], BF16, kind="ExternalInput"
        )
        uw2_d = nc.dram_tensor(
            "uw2", [INTER // 512 * HIDDEN, 512], BF16, kind="ExternalInput"
        )
        dw2_d = nc.dram_tensor("dw2", [INTER, HIDDEN], BF16, kind="ExternalInput")
    y_d = nc.dram_tensor("y", [c_out, HIDDEN], F32, kind="ExternalOutput")

    groups = _token_groups(c_tokens)
    n_tok_tiles = c_out // 128

    with ExitStack() as ctx:
        tc = ctx.enter_context(tile.TileContext(nc))
        wpool = ctx.enter_context(tc.tile_pool(name="weights", bufs=1))
        dwpool = ctx.enter_context(tc.tile_pool(name="dw", bufs=8))
        xpool = ctx.enter_context(tc.tile_pool(name="xt", bufs=2))
        hpool = ctx.enter_context(tc.tile_pool(name="h", bufs=1))
        spool = ctx.enter_context(tc.tile_pool(name="s", bufs=2))
        opool = ctx.enter_context(tc.tile_pool(name="out", bufs=2))
        # one shared 8-slot PSUM pool: phase A cycles gate/up pairs through
        # it (2 live), phase B holds all 8 down-accumulators at once
        pspool = ctx.enter_context(tc.tile_pool(name="ps", bufs=8, space="PSUM"))

        # resident gate/up weights, split into 512-column blocks (separate
        # tiles) so phase A's first m-tiles only depend on block 0's DMA.
        # block b tile layout: k-tile k at col k*512, covering inter columns
        # [b*512, (b+1)*512) of the original [HIDDEN, INTER] weight.
        n_blk = INTER // 512  # 8
        gw_blks = [
            wpool.tile([128, DK * 512], BF16, tag=f"gw{b}", name=f"gwb{b}")
            for b in range(n_blk)
        ]
        uw_blks = [
            wpool.tile([128, DK * 512], BF16, tag=f"uw{b}", name=f"uwb{b}")
            for b in range(n_blk)
        ]

        def load_wblock(b):
            for k in range(DK):
                r0 = b * HIDDEN + k * 128
                nc.sync.dma_start(
                    gw_blks[b][:, k * 512:(k + 1) * 512], gw_d[r0:r0 + 128, :]
                )
                nc.sync.dma_start(
                    uw_blks[b][:, k * 512:(k + 1) * 512], uw_d[r0:r0 + 128, :]
                )

        # group 0's xt interleaved with gate block 0 so the first
        # accumulation group can start after a couple of DMAs
        xt_first = xpool.tile([128, DK * 512], BF16, tag="xt", name="xt_first")
        g0off, g0wid = groups[0]
        for k in range(DK):
            nc.sync.dma_start(
                xt_first[:, k * 512:k * 512 + g0wid],
                xt_d[k * 128:(k + 1) * 128, g0off:g0off + g0wid],
            )
            r0 = k * 128
            nc.sync.dma_start(
                gw_blks[0][:, k * 512:(k + 1) * 512], gw_d[r0:r0 + 128, :]
            )
            nc.sync.dma_start(
                uw_blks[0][:, k * 512:(k + 1) * 512], uw_d[r0:r0 + 128, :]
            )
        for b in range(1, n_blk):
            load_wblock(b)
        # combine weights: column t = token tile t, [128, 1] each (tiny)
        cw_sb = wpool.tile([128, n_tok_tiles], F32, tag="cw")
        for t in range(n_tok_tiles):
            nc.sync.dma_start(cw_sb[:, t:t + 1], cw_d[t * 128:(t + 1) * 128, :])

        for gi, (goff, gwid) in enumerate(groups):
            # ---- load x^T for this token group: k-tile k at col k*512 ----
            if gi == 0:
                xt_sb = xt_first
            else:
                xt_sb = xpool.tile([128, DK * 512], BF16, tag="xt")
                for k in range(DK):
                    nc.sync.dma_start(
                        xt_sb[:, k * 512:k * 512 + gwid],
                        xt_d[k * 128:(k + 1) * 128, goff:goff + gwid],
                    )

            # ---- phase A: h^T[m] = silu(G^T x) * (U^T x), inter on partitions
            h_sb = hpool.tile([128, IM * 512], BF16, tag="h")
            for m in range(IM):
                blk, moff = m // 4, (m % 4) * 128
                pg = pspool.tile([128, gwid], F32, tag="ps")
                pu = pspool.tile([128, gwid], F32, tag="ps")
                for k in range(DK):
                    nc.tensor.matmul(
                        pg[:],
                        gw_blks[blk][:, k * 512 + moff:k * 512 + moff + 128],
                        xt_sb[:, k * 512:k * 512 + gwid],
                        start=(k == 0),
                        stop=(k == DK - 1),
                    )
                for k in range(DK):
                    nc.tensor.matmul(
                        pu[:],
                        uw_blks[blk][:, k * 512 + moff:k * 512 + moff + 128],
                        xt_sb[:, k * 512:k * 512 + gwid],
                        start=(k == 0),
                        stop=(k == DK - 1),
                    )
                s_sb = spool.tile([128, gwid], F32, tag="s")
                nc.scalar.activation(
                    s_sb[:], pg[:], mybir.ActivationFunctionType.Silu
                )
                nc.vector.tensor_mul(
                    h_sb[:, m * 512:m * 512 + gwid], s_sb[:], pu[:]
                )

            # ---- phase B: y[tok] = h^T.T @ Dw, scaled by combine weight ----
            # m outermost so each streamed dw tile is consumed and released;
            # all (half, t) accumulators live across the m loop.
            n_t = gwid // 128
            py_tiles = [
                [
                    pspool.tile([128, 512], F32, tag="ps", name=f"py_{half}_{t}")
                    for t in range(n_t)
                ]
                for half in range(2)
            ]
            for m in range(IM):
                dwt = dwpool.tile([128, HIDDEN], BF16, tag="dw")
                nc.sync.dma_start(dwt[:], dw_d[m * 128:(m + 1) * 128, :])
                for half in range(2):
                    for t in range(n_t):
                        nc.tensor.matmul(
                            py_tiles[half][t][:],
                            h_sb[:, m * 512 + t * 128:m * 512 + (t + 1) * 128],
                            dwt[:, half * 512:(half + 1) * 512],
                            start=(m == 0),
                            stop=(m == IM - 1),
                        )
            for t in range(n_t):
                out_sb = opool.tile([128, HIDDEN], F32, tag="out")
                tile_idx = goff // 128 + t
                for half in range(2):
                    nc.vector.tensor_scalar_mul(
                        out_sb[:, half * 512:(half + 1) * 512],
                        py_tiles[half][t][:],
                        cw_sb[:, tile_idx:tile_idx + 1],
                    )
                nc.sync.dma_start(
                    y_d[tile_idx * 128:(tile_idx + 1) * 128, :], out_sb[:]
                )

        if secondary:
            # ---- secondary expert: one 128-token tile.  Weight blocks are
            # allocated into the same tags as the main weights, so their
            # DMAs stream in as the last main group releases each slot.
            xt2_sb = wpool.tile([128, DK * 128], BF16, tag="xt2")
            for k in range(DK):
                nc.sync.dma_start(
                    xt2_sb[:, k * 128:(k + 1) * 128], xt2_d[k * 128:(k + 1) * 128, :]
                )
            gw2_blks = [
                wpool.tile([128, DK * 512], BF16, tag=f"gw{b}", name=f"gw2b{b}")
                for b in range(n_blk)
            ]
            uw2_blks = [
                wpool.tile([128, DK * 512], BF16, tag=f"uw{b}", name=f"uw2b{b}")
                for b in range(n_blk)
            ]
            for b in range(n_blk):
                for k in range(DK):
                    r0 = b * HIDDEN + k * 128
                    nc.sync.dma_start(
                        gw2_blks[b][:, k * 512:(k + 1) * 512], gw2_d[r0:r0 + 128, :]
                    )
                    nc.sync.dma_start(
                        uw2_blks[b][:, k * 512:(k + 1) * 512], uw2_d[r0:r0 + 128, :]
                    )

            h2_sb = hpool.tile([128, IM * 512], BF16, tag="h", name="h2")
            for m in range(IM):
                blk, moff = m // 4, (m % 4) * 128
                pg = pspool.tile([128, 128], F32, tag="ps", name="pg2")
                pu = pspool.tile([128, 128], F32, tag="ps", name="pu2")
                for k in range(DK):
                    nc.tensor.matmul(
                        pg[:],
                        gw2_blks[blk][:, k * 512 + moff:k * 512 + moff + 128],
                        xt2_sb[:, k * 128:(k + 1) * 128],
                        start=(k == 0),
                        stop=(k == DK - 1),
                    )
                for k in range(DK):
                    nc.tensor.matmul(
                        pu[:],
                        uw2_blks[blk][:, k * 512 + moff:k * 512 + moff + 128],
                        xt2_sb[:, k * 128:(k + 1) * 128],
                        start=(k == 0),
                        stop=(k == DK - 1),
                    )
                s2_sb = spool.tile([128, 128], F32, tag="s", name="s2")
                nc.scalar.activation(
                    s2_sb[:], pg[:], mybir.ActivationFunctionType.Silu
                )
                nc.vector.tensor_mul(
                    h2_sb[:, m * 512:m * 512 + 128], s2_sb[:], pu[:]
                )

            # pre-stage all dw2 m-tiles into the gw2/uw2 block slots (dead
            # after phase A2 reads them) so B2 isn't DMA-bound
            dw2pre = [
                wpool.tile(
                    [128, 4 * HIDDEN],
                    BF16,
                    tag=(f"gw{j}" if j < 4 else f"uw{j - 4}"),
                    name=f"dw2pre{j}",
                )
                for j in range(8)
            ]
            for j in range(8):
                for i in range(4):
                    m = j * 4 + i
                    nc.sync.dma_start(
                        dw2pre[j][:, i * HIDDEN:(i + 1) * HIDDEN],
                        dw2_d[m * 128:(m + 1) * 128, :],
                    )

            py2 = [
                pspool.tile([128, 512], F32, tag="ps", name=f"py2_{half}")
                for half in range(2)
            ]
            for m in range(IM):
                base = (m % 4) * HIDDEN
                halves = [
                    dw2pre[m // 4][:, base + h * 512:base + (h + 1) * 512]
                    for h in range(2)
                ]
                for half in range(2):
                    nc.tensor.matmul(
                        py2[half][:],
                        h2_sb[:, m * 512:m * 512 + 128],
                        halves[half],
                        start=(m == 0),
                        stop=(m == IM - 1),
                    )
            t2 = c_tokens // 128
            out2_sb = opool.tile([128, HIDDEN], F32, tag="out", name="out2")
            for half in range(2):
                nc.vector.tensor_scalar_mul(
                    out2_sb[:, half * 512:(half + 1) * 512],
                    py2[half][:],
                    cw_sb[:, t2:t2 + 1],
                )
            nc.sync.dma_start(y_d[c_tokens:c_tokens + 128, :], out2_sb[:])

    _split_multi_waits(nc)
    return nc


@functools.lru_cache(maxsize=2)
def _compiled_ncs(c_tokens, secondary=False):
    return build_expert_kernel(c_tokens, secondary)


def _route_host(x, router_w):
    """fp32 host routing identical to the jax reference."""
    logits = x @ router_w                                     # [T, E]
    m = logits.max(axis=-1, keepdims=True)
    p = np.exp(logits - m)
    p /= p.sum(axis=-1, keepdims=True)
    idx = np.argsort(-p, axis=-1, kind="stable")[:, :TOP_K]   # [T, K]
    w = np.take_along_axis(p, idx, axis=-1)
    w = w / w.sum(axis=-1, keepdims=True)
    return p, idx, w


def kernel(hidden_states, router_w, gate_w, up_w, down_w):
    global last_results
    hidden_states = np.asarray(hidden_states, dtype=np.float32)
    router_w = np.asarray(router_w, dtype=np.float32)
    x = hidden_states.reshape(T, HIDDEN)

    probs, idx, w = _route_host(x, router_w)

    # load-balance loss (host, fp32, matches reference formula)
    counts = np.bincount(idx.ravel(), minlength=NUM_EXPERTS)
    expert_usage = counts.astype(np.float32) / np.float32(T)
    mean_probs = probs.mean(axis=0, dtype=np.float32)
    loss = np.float32((expert_usage * mean_probs).sum() * NUM_EXPERTS)

    # capacity: padded max expert load (deterministic inputs -> 2304)
    c_tokens = max(2304, int(-(-counts.max() // 128) * 128))

    xb = x.astype(ml_dtypes.bfloat16)
    # pre-block gate/up to [E, INTER//512, HIDDEN, 512] -> rows contiguous
    gwb = (
        np.asarray(gate_w)
        .astype(ml_dtypes.bfloat16)
        .reshape(NUM_EXPERTS, HIDDEN, INTER // 512, 512)
        .transpose(0, 2, 1, 3)
        .reshape(NUM_EXPERTS, INTER // 512 * HIDDEN, 512)
    )
    uwb = (
        np.asarray(up_w)
        .astype(ml_dtypes.bfloat16)
        .reshape(NUM_EXPERTS, HIDDEN, INTER // 512, 512)
        .transpose(0, 2, 1, 3)
        .reshape(NUM_EXPERTS, INTER // 512 * HIDDEN, 512)
    )
    dwb = np.asarray(down_w).astype(ml_dtypes.bfloat16)

    routed = []
    for e in range(NUM_EXPERTS):
        tok_e, k_e = np.nonzero(idx == e)
        routed.append((tok_e, k_e))

    # ---- balanced plan: 2048 main tokens/core + one <=128-token secondary
    # tile carrying another expert's overflow (with replicated weights) ----
    T1 = 2048
    chunks = []
    for e in range(NUM_EXPERTS):
        tok_e, k_e = routed[e]
        for i in range(T1, len(tok_e), 128):
            chunks.append((e, tok_e[i:i + 128], k_e[i:i + 128]))
    balanced = len(chunks) <= NUM_EXPERTS

    if balanced:
        c_main = T1
        in_maps = []
        for e in range(NUM_EXPERTS):
            tok_e, k_e = routed[e]
            n_main = min(len(tok_e), T1)
            xt = np.zeros((HIDDEN, c_main), dtype=ml_dtypes.bfloat16)
            xt[:, :n_main] = xb[tok_e[:n_main]].T
            cw = np.zeros((c_main + 128, 1), dtype=np.float32)
            cw[:n_main, 0] = w[tok_e[:n_main], k_e[:n_main]]
            m = {
                "xt": xt,
                "gw": np.ascontiguousarray(gwb[e]),
                "uw": np.ascontiguousarray(uwb[e]),
                "dw": np.ascontiguousarray(dwb[e]),
                "cw": cw,
            }
            if e < len(chunks):
                e2, tok2, k2 = chunks[e]
                n2 = len(tok2)
                xt2 = np.zeros((HIDDEN, 128), dtype=ml_dtypes.bfloat16)
                xt2[:, :n2] = xb[tok2].T
                cw[c_main:c_main + n2, 0] = w[tok2, k2]
                m["xt2"] = xt2
                m["gw2"] = np.ascontiguousarray(gwb[e2])
                m["uw2"] = np.ascontiguousarray(uwb[e2])
                m["dw2"] = np.ascontiguousarray(dwb[e2])
            else:
                zw = np.zeros((INTER // 512 * HIDDEN, 512), dtype=ml_dtypes.bfloat16)
                m["xt2"] = np.zeros((HIDDEN, 128), dtype=ml_dtypes.bfloat16)
                m["gw2"] = zw
                m["uw2"] = zw
                m["dw2"] = np.zeros((INTER, HIDDEN), dtype=ml_dtypes.bfloat16)
            in_maps.append(m)

        nc = _compiled_ncs(c_main, True)
        res = run_bass_kernel_spmd(nc, in_maps, core_ids=list(range(N_CORES)))
        last_results = res

        out = np.zeros((T, HIDDEN), dtype=np.float32)
        for e in range(NUM_EXPERTS):
            tok_e, k_e = routed[e]
            n_main = min(len(tok_e), T1)
            out[tok_e[:n_main]] += res.results[e]["y"][:n_main]
            if e < len(chunks):
                _, tok2, _ = chunks[e]
                out[tok2] += res.results[e]["y"][c_main:c_main + len(tok2)]
        return out.reshape(B, S, HIDDEN), loss

    # ---- fallback: pure per-expert padding ----
    in_maps = []
    for e in range(NUM_EXPERTS):
        tok_e, k_e = routed[e]
        n_e = len(tok_e)
        xt = np.zeros((HIDDEN, c_tokens), dtype=ml_dtypes.bfloat16)
        xt[:, :n_e] = xb[tok_e].T
        cw = np.zeros((c_tokens, 1), dtype=np.float32)
        cw[:n_e, 0] = w[tok_e, k_e]
        in_maps.append(
            {
                "xt": xt,
                "gw": np.ascontiguousarray(gwb[e]),
                "uw": np.ascontiguousarray(uwb[e]),
                "dw": np.ascontiguousarray(dwb[e]),
                "cw": cw,
            }
        )

    nc = _compiled_ncs(c_tokens)
    res = run_bass_kernel_spmd(nc, in_maps, core_ids=list(range(N_CORES)))
    last_results = res

    out = np.zeros((T, HIDDEN), dtype=np.float32)
    for e in range(NUM_EXPERTS):
        tok_e, _ = routed[e]
        out[tok_e] += res.results[e]["y"][:len(tok_e)]

    return out.reshape(B, S, HIDDEN), loss
